# revision 1
# baseline (speedup 1.0000x reference)
"""CartBasisStressHead kernel for Trainium2 (8 NeuronCores, SPMD data-parallel).

Strategy
--------
Only 6 of the 9 m-rows of node_embedding are used: row 0 feeds a SiLU MLP
(per-node scalar), rows 4:9 feed a per-channel contraction (l=2 branch).
Nodes are sharded contiguously across 8 cores; segment sums are linear, so
the host adds partials for graphs that straddle shard/group boundaries.

The kernel is HBM-bandwidth-bound, so everything rides the wire in fp8-e4m3:

* l=2 rows are quantized with error feedback along the node axis (carry
  reset at each 1024-node group, the PSUM accumulation span), so the
  device-side segment sum telescopes the rounding error: per-graph partial
  sums see ~1 ulp of error instead of sqrt(n)*ulp. Measured end-to-end
  max-rel error ~6e-3 (gate 2e-2).
* The MLP input x0 is plain fp8; W1 is decomposed into hi+lo fp8 parts
  contracted in a single DoubleRow (double-pumped fp8) matmul with a
  stride-0 k-tile on the moving x0, recovering ~bf16 weight precision.
* The segment-sum matmul streams el2 through single-pump fp8 matmuls with
  a host-built 0/1 indicator matrix A[node, local_graph] as the stationary
  operand, spread over all four PE column quadrants (tiles 0-3 on q0/q32,
  tiles 4-7 on q64/q96, two accumulators recombined on host).
* The w_l2 channel contraction runs on-device (vector-engine multiply by a
  replicated-wl2 constant + 64-chunk reduce straight out of PSUM), so only
  [rows, 5] partials ship to the host instead of the full (m,c) matrix.

Per group (1024 nodes): 2 input DMAs (x0 1KB/partition + el2 5KB/partition,
sync queue), 6 MLP matmuls + 16 aniso matmuls, 2 merged [128,1024] SiLUs
(scalar engine, 2-bank PSUM reads), 1 wide PSUM->SBUF copy + mult/reduce
(vector engine), outputs batched over 4 groups on the gpsimd queue. The MLP
is software-pipelined across groups (L2 lags one group, w3 two) so the
in-order Tensor queue never waits on a SiLU.

Host epilogue: bincount segment-sum of per-node scalars, scatter-add of
per-group aniso partials, and the tiny (G,9)@(9,9) change-of-basis.
"""

import sys

if "/opt/trn_rl_repo" not in sys.path:
    sys.path.insert(0, "/opt/trn_rl_repo")

import numpy as np
import ml_dtypes

import concourse.bacc as bacc
import concourse.tile as tile
from concourse import mybir
from concourse import bass_utils

_S2 = 2.0 ** -0.5
_S3 = 3.0 ** -0.5
_S6 = 6.0 ** -0.5
_CG = np.array([
    [_S3, 0, 0, 0, _S3, 0, 0, 0, _S3],
    [0, 0, 0, 0, 0, _S2, 0, -_S2, 0],
    [0, 0, -_S2, 0, 0, 0, _S2, 0, 0],
    [0, _S2, 0, -_S2, 0, 0, 0, 0, 0],
    [0, 0, _S2, 0, 0, 0, _S2, 0, 0],
    [0, 0, 0, 0, 0, _S2, 0, _S2, 0],
    [-_S6, 0, 0, 0, 2 * _S6, 0, 0, 0, -_S6],
    [0, _S2, 0, _S2, 0, 0, 0, 0, 0],
    [-_S2, 0, 0, 0, 0, 0, 0, 0, _S2],
], dtype=np.float32)  # (9, 9)

N_CORES = 8
P = 128          # SBUF partitions
NG = 1024        # nodes per group (one PSUM accumulation span)
NB = 256         # nodes per DoubleRow block (2 k-tiles of 128)
BLK = NG // NB   # blocks per group
ML2 = 5 * P      # 640 values of l=2 data per node
OB = 4           # groups per output staging batch

F32 = mybir.dt.float32
BF16 = mybir.dt.bfloat16
FP8 = mybir.dt.float8e4
WIRE8 = ml_dtypes.float8_e4m3
WIRE16 = ml_dtypes.bfloat16

_BUILD_CACHE = {}


def _build(n_pad, n_groups, W, n_real):
    key = (n_pad, n_groups, W, n_real)
    if key in _BUILD_CACHE:
        return _BUILD_CACHE[key]

    n_ob = (n_groups + OB - 1) // OB
    dr = mybir.MatmulPerfMode.DoubleRow
    silu = mybir.ActivationFunctionType.Silu
    W2r = 32                 # row offset of the second 320-col half

    nc = bacc.Bacc("TRN2", target_bir_lowering=False, debug=False,
                   num_devices=N_CORES)

    x0T = nc.dram_tensor("x0T", (P, n_pad), FP8, kind="ExternalInput").ap()
    # host pre-tiled: [group, p, (blk, kt, m)] contiguous per partition
    embL2 = nc.dram_tensor("embL2", (n_groups, P, BLK * 2 * ML2), FP8,
                           kind="ExternalInput").ap()
    # host-built indicator: [p, (group, blk, kt, W)]
    A_in = nc.dram_tensor("A_in", (P, n_groups * BLK * 2 * W), FP8,
                          kind="ExternalInput").ap()
    w1hl = nc.dram_tensor("w1hl", (P, 2 * P), FP8, kind="ExternalInput").ap()
    w2t = nc.dram_tensor("w2t", (P, P), BF16, kind="ExternalInput").ap()
    w3t = nc.dram_tensor("w3t", (P, 1), BF16, kind="ExternalInput").ap()
    b1 = nc.dram_tensor("b1c", (P, 1), F32, kind="ExternalInput").ap()
    b2 = nc.dram_tensor("b2c", (P, 1), F32, kind="ExternalInput").ap()
    scal = nc.dram_tensor("scal", (n_ob, 2, OB * 512), F32,
                          kind="ExternalOutput").ap()
    S_out = nc.dram_tensor("S_out", (n_ob, 96 + W, OB * 5), F32,
                           kind="ExternalOutput").ap()
    wl2rep = nc.dram_tensor("wl2rep", (P, 320), F32,
                            kind="ExternalInput").ap()

    with tile.TileContext(nc) as tc:
        with (
            tc.tile_pool(name="const", bufs=1) as cpool,
            tc.tile_pool(name="x0p", bufs=16) as x0p,
            tc.tile_pool(name="el2p", bufs=16) as el2p,
            tc.tile_pool(name="hp", bufs=8) as hp,
            tc.tile_pool(name="stp", bufs=6) as stp,
            tc.tile_pool(name="ph1", bufs=1, space="PSUM") as ph1p,
            tc.tile_pool(name="ph2", bufs=1, space="PSUM") as ph2p,
            tc.tile_pool(name="psc", bufs=2, space="PSUM") as pscp,
            tc.tile_pool(name="pS", bufs=2, space="PSUM") as pSp,
        ):
            w1s = cpool.tile([P, 2 * P], FP8)
            w2s = cpool.tile([P, P], BF16)
            w3s = cpool.tile([P, 1], BF16)
            b1s = cpool.tile([P, 1], F32)
            b2s = cpool.tile([P, 1], F32)
            Aall = cpool.tile([P, n_groups * BLK * 2 * W], FP8)
            wl2s = cpool.tile([P, 320], F32)
            nc.scalar.dma_start(out=wl2s[:], in_=wl2rep)
            nc.sync.dma_start(out=w1s[:], in_=w1hl)
            nc.scalar.dma_start(out=w2s[:], in_=w2t)
            nc.scalar.dma_start(out=w3s[:], in_=w3t)
            nc.sync.dma_start(out=b1s[:], in_=b1)
            nc.scalar.dma_start(out=b2s[:], in_=b2)
            nc.scalar.dma_start(out=Aall[:], in_=A_in)

            # Cross-group software pipeline: the in-order Tensor queue
            # never waits on a SiLU. Per iteration g it runs L1(g),
            # L2(g-1), w3(g-2), aniso(g); SiLUs for (g) and (g-1) land on
            # the scalar engine in between, copies trail on the vector
            # engine.
            h1l = {}   # g -> [h1s chunks]
            h2l = {}   # g -> [h2s chunks]
            scl = {}   # g -> scp psum tile
            srl = {}   # g -> Sr
            scst = Sst = None
            scst_by_g = {}

            def stage_l1(g):
                grp_real = min(NG, n_real - g * NG)
                Sr = (grp_real + 511) // 512
                srl[g] = Sr
                x0c = x0p.tile([P, NG], FP8, tag="x0c")
                nc.sync.dma_start(
                    out=x0c[:, :Sr * 512],
                    in_=x0T[:, g * NG: g * NG + Sr * 512])
                el2c = el2p.tile([P, BLK * 2 * ML2], FP8, tag="el2c")
                Bl = (grp_real + NB - 1) // NB
                nc.sync.dma_start(
                    out=el2c[:, :Bl * 2 * ML2],
                    in_=embL2[g][:, :Bl * 2 * ML2])
                h1p = ph1p.tile([P, NG], F32, tag="h1p")
                for s in range(Sr):
                    nsl = slice(s * 512, (s + 1) * 512)
                    nc.tensor.matmul(
                        h1p[:, nsl],
                        w1s[:].rearrange("p (k h) -> p k h", k=2, h=P),
                        x0c[:, nsl].unsqueeze(1).to_broadcast([P, 2, 512]),
                        start=True, stop=True, perf_mode=dr)
                h1s = hp.tile([P, NG], BF16, tag="h1s")
                nc.scalar.activation(h1s[:, :Sr * 512], h1p[:, :Sr * 512],
                                     silu, bias=b1s[:])
                h1l[g] = h1s
                return el2c, Bl

            def stage_l2(g):
                Sr = srl[g]
                h2p = ph2p.tile([P, NG], F32, tag="h2p")
                for s in range(Sr):
                    nsl = slice(s * 512, (s + 1) * 512)
                    nc.tensor.matmul(h2p[:, nsl], w2s[:], h1l[g][:, nsl],
                                     start=True, stop=True)
                h2s = hp.tile([P, NG], BF16, tag="h2s")
                nc.scalar.activation(h2s[:, :Sr * 512], h2p[:, :Sr * 512],
                                     silu, bias=b2s[:])
                h2l[g] = h2s
                del h1l[g]

            def stage_w3(g):
                nonlocal scst
                if g % OB == 0:
                    scst = stp.tile([P, OB * 512], F32, tag="scst")
                scst_by_g[g] = scst
                scp = pscp.tile([P, 512], F32, tag="scp")
                scl[g] = scp
                for s in range(srl[g]):
                    q = 64 + 32 * s
                    nc.tensor.matmul(scp[q:q + 1, :], w3s[:],
                                     h2l[g][:, s * 512:(s + 1) * 512],
                                     start=True, stop=True,
                                     tile_position=(0, q))
                del h2l[g]

            def stage_scst(g):
                boff = g % OB
                scp = scl.pop(g)
                nc.vector.tensor_copy(
                    out=scst_by_g[g][64:97, boff * 512:(boff + 1) * 512],
                    in_=scp[64:97, :])
                if g % OB == OB - 1 or g == n_groups - 1:
                    nc.gpsimd.dma_start(out=scal[g // OB, 0],
                                        in_=scst_by_g[g][64:65, :])
                    nc.gpsimd.dma_start(out=scal[g // OB, 1],
                                        in_=scst_by_g[g][96:97, :])
                del scst_by_g[g]

            for grp in range(n_groups):
                el2c, Bl = stage_l1(grp)
                if grp >= 1:
                    stage_l2(grp - 1)
                if grp >= 2:
                    stage_w3(grp - 2)
                    stage_scst(grp - 2)

                # ---- l=2 branch: fp8 segment-sum matmuls on two PE
                # column groups (q0/q32) ----
                if grp % OB == 0:
                    Sst = stp.tile([96 + W, OB * 5], F32, tag="SstR")
                boff = grp % OB
                pS4 = pSp.tile([96 + W, 320], F32, tag="pS4")
                Abase = grp * BLK * 2 * W
                Tl = 2 * Bl
                Ha = min(Tl, 4)
                for t in range(Tl):
                    As = Aall[:, Abase + t * W: Abase + (t + 1) * W]
                    ebase = t * ML2
                    ro = 0 if t < 4 else 64
                    st = t == 0 if t < 4 else t == 4
                    sp = t == Ha - 1 if t < 4 else t == Tl - 1
                    nc.tensor.matmul(pS4[ro:ro + W, :], As,
                                     el2c[:, ebase: ebase + 320],
                                     start=st, stop=sp,
                                     tile_position=(0, ro))
                    nc.tensor.matmul(pS4[ro + 32:ro + 32 + W, :], As,
                                     el2c[:, ebase + 320: ebase + ML2],
                                     start=st, stop=sp,
                                     tile_position=(0, ro + 32))
                Sml = stp.tile([96 + W, 320], F32, tag="Sml")
                nc.vector.tensor_tensor(out=Sml[:], in0=pS4[:],
                                        in1=wl2s[:96 + W, :],
                                        op=mybir.AluOpType.mult)
                nc.vector.tensor_reduce(
                    out=Sst[:, boff * 5:(boff + 1) * 5],
                    in_=Sml[:].rearrange("p (f c) -> p f c", f=5, c=64),
                    axis=mybir.AxisListType.X,
                    op=mybir.AluOpType.add)
                if grp % OB == OB - 1 or grp == n_groups - 1:
                    nc.gpsimd.dma_start(out=S_out[grp // OB], in_=Sst[:])

            # pipeline flush
            if (n_groups - 1) in h1l:
                stage_l2(n_groups - 1)
            for g in (n_groups - 2, n_groups - 1):
                if g >= 0 and g in h2l:
                    stage_w3(g)
                    stage_scst(g)
    nc.compile()
    _BUILD_CACHE[key] = nc
    return nc


def _next_pow2(x):
    p = 16
    while p < x:
        p *= 2
    return p


def _fb_quant_groups(x, span):
    """Error-feedback fp8 quantization along axis 1 (node-in-group axis).

    x: (n_fibers, span, feat) float32. The carry telescopes rounding error
    along each group's node scan so segment sums of the quantized values
    track the exact sums to ~1 ulp.
    """
    q = np.empty(x.shape, WIRE8)
    carry = np.zeros((x.shape[0], x.shape[2]), np.float32)
    for i in range(span):
        carry += x[:, i, :]
        qi = carry.astype(WIRE8)
        q[:, i, :] = qi
        carry -= qi.astype(np.float32)
    return q


def _host_reference(node_embedding, W1, b1, W2, b2, W3, b3, w_l2, batch,
                    natoms):
    """Pure-numpy fallback (only used for pathological graph layouts)."""
    G = natoms.shape[0]
    inv = 1.0 / natoms.astype(np.float32)
    x = node_embedding[:, 0, :]
    h = x @ W1.T + b1
    h = h / (1.0 + np.exp(-h))
    h = h @ W2.T + b2
    h = h / (1.0 + np.exp(-h))
    ns = (h @ W3.T + b3)[:, 0]
    ok = (batch >= 0) & (batch < G)
    bok = batch[ok]
    iso = np.bincount(bok, weights=ns[ok], minlength=G).astype(np.float32) \
        * inv
    nl2 = np.einsum("nmc,c->nm", node_embedding[:, 4:9, :], w_l2[0])
    aniso = np.stack(
        [np.bincount(bok, weights=nl2[ok, m], minlength=G)
         for m in range(5)], axis=1).astype(np.float32) * inv[:, None]
    dec = np.concatenate([iso[:, None], np.zeros((G, 3), np.float32), aniso],
                         axis=1)
    return (dec @ _CG).reshape(-1, 3, 3).astype(np.float32)


def kernel(node_embedding, W1, b1, W2, b2, W3, b3, w_l2, batch, natoms):
    node_embedding = np.asarray(node_embedding, dtype=np.float32)
    W1 = np.asarray(W1, dtype=np.float32)
    b1 = np.asarray(b1, dtype=np.float32)
    W2 = np.asarray(W2, dtype=np.float32)
    b2 = np.asarray(b2, dtype=np.float32)
    W3 = np.asarray(W3, dtype=np.float32)
    b3 = np.asarray(b3, dtype=np.float32)
    w_l2 = np.asarray(w_l2, dtype=np.float32)
    batch = np.asarray(batch).astype(np.int64)
    natoms_in = np.asarray(natoms)

    N = node_embedding.shape[0]
    G = natoms_in.shape[0]
    n_sh = (N + N_CORES - 1) // N_CORES
    n_groups = (n_sh + NG - 1) // NG
    n_pad = n_groups * NG

    # per-core shard ranges and group graph bases
    shards = []
    W_need = 16
    for c in range(N_CORES):
        n0 = min(c * n_sh, N)
        n1 = min(n0 + n_sh, N)
        b = batch[n0:n1]
        nreal = n1 - n0
        gbase = np.zeros(n_groups, np.int64)
        for grp in range(n_groups):
            lo = grp * NG
            hi = min(lo + NG, nreal)
            if lo < nreal:
                gbase[grp] = b[lo]
                span = int(b[hi - 1] - b[lo] + 1)
                W_need = max(W_need, span)
        shards.append((n0, n1, b, gbase))
    W = _next_pow2(W_need)
    if (W > 32 or not np.all(batch[:-1] <= batch[1:])
            or batch.min(initial=0) < 0 or batch.max(initial=0) >= G):
        return _host_reference(node_embedding, W1, b1, W2, b2, W3, b3,
                               w_l2, batch, natoms_in)
    W2r = 32

    nc = _build(n_pad, n_groups, W, n_sh)

    W1hi = W1.astype(WIRE8).astype(np.float32)
    W1lo = (W1 - W1hi).astype(WIRE8)
    w1hl = np.ascontiguousarray(
        np.stack([W1hi.astype(WIRE8).T, W1lo.T], axis=1)
        .reshape(P, 2 * P)).astype(WIRE8)
    w2t = np.ascontiguousarray(W2.T).astype(WIRE16)
    w3t = np.ascontiguousarray(W3.T).astype(WIRE16)
    b1c = np.ascontiguousarray(b1[:, None])
    b2c = np.ascontiguousarray(b2[:, None])

    # wl2 replication constant: rows 0:48 cover flat (m,c) 0:320
    # (c = j % 128), rows 64:112 cover 320:640 (c = (64 + j) % 128)
    wl2rep = np.zeros((P, 320), np.float32)
    patA = w_l2[0][np.arange(320) % 128]
    patB = w_l2[0][(64 + np.arange(320)) % 128]
    wl2rep[0:32] = patA
    wl2rep[32:64] = patB
    wl2rep[64:96] = patA
    wl2rep[96:128] = patB

    # error-feedback quantize the l=2 rows for all cores at once:
    # fibers = (core, group), scan axis = node within group
    el2_all = np.zeros((N_CORES, n_pad, ML2), np.float32)
    for c in range(N_CORES):
        n0, n1, _, _ = shards[c]
        el2_all[c, :n1 - n0] = node_embedding[n0:n1, 4:9, :] \
            .reshape(n1 - n0, ML2)
    el2q = _fb_quant_groups(
        el2_all.reshape(N_CORES * n_groups, NG, ML2), NG) \
        .reshape(N_CORES, n_pad, ML2)

    in_maps = []
    for c in range(N_CORES):
        n0, n1, b, gbase = shards[c]
        nreal = n1 - n0
        x0T = np.zeros((P, n_pad), WIRE8)
        x0T[:, :nreal] = node_embedding[n0:n1, 0, :].T.astype(WIRE8)
        # node = grp*1024 + blk*256 + kt*128 + p
        el2 = np.ascontiguousarray(
            el2q[c].reshape(n_groups, BLK, 2, P, ML2)
            .transpose(0, 3, 1, 2, 4).reshape(n_groups, P, BLK * 2 * ML2))
        lg = np.full(n_pad, -1.0, np.float32)
        lg[:nreal] = (b - np.repeat(gbase, NG)[:nreal]).astype(np.float32)
        A = (lg.reshape(n_groups, BLK, 2, P)[..., None]
             == np.arange(W, dtype=np.float32)).astype(WIRE8)
        A = np.ascontiguousarray(
            A.transpose(3, 0, 1, 2, 4).reshape(P, n_groups * BLK * 2 * W))
        in_maps.append({
            "x0T": x0T, "embL2": el2, "A_in": A, "wl2rep": wl2rep,
            "w1hl": w1hl, "w2t": w2t, "w3t": w3t, "b1c": b1c, "b2c": b2c,
        })

    res = bass_utils.run_bass_kernel_spmd(nc, in_maps,
                                          core_ids=list(range(N_CORES)))

    # ---- host epilogue ----
    inv = (1.0 / natoms_in.astype(np.float32)).astype(np.float32)
    n_ob = (n_groups + OB - 1) // OB
    node_scalar = np.empty(N, np.float32)
    Afull = np.zeros((G + 2 * W, 5), np.float32)
    for c in range(N_CORES):
        n0, n1, _, gbase = shards[c]
        nreal = n1 - n0
        sc = res.results[c]["scal"].reshape(n_ob, 2, OB, 512) \
            .transpose(0, 2, 1, 3).reshape(-1)[:nreal]
        node_scalar[n0:n1] = sc
        Sc = res.results[c]["S_out"]
        for grp in range(n_groups):
            if grp * NG < nreal:
                gb = int(gbase[grp])
                j = grp % OB
                blk = Sc[grp // OB][:, j * 5:(j + 1) * 5]
                qA = blk[0:W]
                qB = blk[32:32 + W]
                if min(NG, nreal - grp * NG) > 512:
                    qA = qA + blk[64:64 + W]
                    qB = qB + blk[96:96 + W]
                av = np.empty((W, 5), np.float32)
                av[:, 0] = qA[:, 0] + qA[:, 1]
                av[:, 1] = qA[:, 2] + qA[:, 3]
                av[:, 2] = qA[:, 4] + qB[:, 0]
                av[:, 3] = qB[:, 1] + qB[:, 2]
                av[:, 4] = qB[:, 3] + qB[:, 4]
                Afull[gb:gb + W] += av
    iso = np.bincount(batch, weights=node_scalar + b3[0], minlength=G)
    iso = iso.astype(np.float32) * inv
    aniso = Afull[:G] * inv[:, None]
    dec = np.concatenate([iso[:, None], np.zeros((G, 3), np.float32), aniso],
                         axis=1)
    return (dec @ _CG).reshape(-1, 3, 3).astype(np.float32)

